# revision 1
# baseline (speedup 1.0000x reference)
"""GCN encoder (3x GCNConv) Trainium2 Bass kernel, 8-core SPMD.

Strategy (dst-sharded message passing):
- Nodes dst-sharded across 8 cores (12544-row padded shards). Each core owns
  all edges (incl. self-loops) whose dst lands in its shard.
- Activations are kept as T' = dis * (H @ W) in fp16, replicated in DRAM via
  AllGather after each layer's transform.
- Propagate per core: for each 128-dst block, gather T'[src] rows via
  gpsimd.dma_gather (int16 indices => T_full split into 4 row-chunks), build
  a one-hot routing matrix oh[e,d] = (dstl[e]==d)*dis[dst_e] on VectorE
  (iota tensor_scalar), and segment-sum with TensorE matmuls accumulating in
  PSUM. dis[src] rides in T' rows, dis[dst] rides in the one-hot, so
  evacuation is a single activation op.
- Layers 1-2 use psum[f,d] = msg.T @ oh so the Relu+bias evac output hT is
  directly the lhsT of the next transform GEMM (no transposes anywhere).
- Layer 3 uses psum[d,f] = oh.T @ msg (+ ones x b3 matmul for bias) and
  writes fp32 node-major output.
"""

import sys
import numpy as np

for _p in ("/opt/trn_rl_repo", "/root/.axon_site/_ro/trn_rl_repo"):
    if _p not in sys.path:
        sys.path.append(_p)

N_NODES = 100000
N_FEAT = 4
D = 128
NC = 8
NCHUNK = 4
GBLK = 8  # blocks per gather group
MAXIDX = 1024  # max indices per dma_gather call (SWDGE desc ring capacity)

f16 = np.float16


# ---------------------------------------------------------------- host side


def _cfg(n_nodes):
    nshard = (n_nodes + NC - 1) // NC
    shpad = ((nshard + 127) // 128) * 128
    nblk = shpad // 128
    nfull = NC * shpad
    assert nfull % NCHUNK == 0
    chunk = nfull // NCHUNK
    assert chunk <= 32767 + 1  # int16 index reach (idx < chunk <= 32768)
    return dict(n=n_nodes, nshard=nshard, shpad=shpad, nblk=nblk,
                nfull=nfull, chunk=chunk)


def _groups(nblk):
    return [(g, min(g + GBLK, nblk)) for g in range(0, nblk, GBLK)]


def _build_schedule(cfg, edge_index):
    """Integer/index preprocessing. Returns shared capacities + per-core
    slot arrays (idx int16, dstl f16, degdst f32) and per-core deg layout."""
    n, nshard, shpad, nblk, chunk = (cfg[k] for k in
                                     ("n", "nshard", "shpad", "nblk", "chunk"))
    src = np.concatenate([edge_index[0], np.arange(n)]).astype(np.int64)
    dst = np.concatenate([edge_index[1], np.arange(n)]).astype(np.int64)
    deg = np.bincount(dst, minlength=n).astype(np.int64)

    rows = (src // nshard) * shpad + (src % nshard)  # row in T_full layout
    echunk = rows // chunk
    ecore = dst // nshard
    eblk = (dst % nshard) // 128
    edstl = (dst % nshard) % 128

    counts = np.zeros((NC, NCHUNK, nblk), dtype=np.int64)
    np.add.at(counts, (ecore, echunk, eblk), 1)
    cap = counts.max(axis=0)
    cap = np.maximum(((cap + 127) // 128) * 128, 128)  # [NCHUNK, nblk]

    # static slot offsets matching emission order: group -> chunk -> block.
    # Each (group, chunk) region is subdivided into gather calls of at most
    # MAXIDX slots (SWDGE descriptor-ring capacity), never splitting a block.
    offs = np.zeros((NCHUNK, nblk), dtype=np.int64)
    calls = []  # (chunk, group_index, slot_off, nslots) per gather call
    off = 0
    for gi, (blo, bhi) in enumerate(_groups(nblk)):
        for c in range(NCHUNK):
            call_off = off
            for b in range(blo, bhi):
                if off + int(cap[c, b]) - call_off > MAXIDX:
                    calls.append((c, gi, call_off, off - call_off))
                    call_off = off
                offs[c, b] = off
                off += int(cap[c, b])
            calls.append((c, gi, call_off, off - call_off))
    calls = [cl for cl in calls if cl[3] > 0]
    total = off

    cores = []
    for ci in range(NC):
        m = ecore == ci
        r, ec, eb, dl, dd = rows[m], echunk[m], eblk[m], edstl[m], dst[m]
        order = np.lexsort((r, eb, ec))
        r, ec, eb, dl, dd = (a[order] for a in (r, ec, eb, dl, dd))
        key = ec * nblk + eb
        starts = np.searchsorted(key, np.arange(NCHUNK * nblk))
        ends = np.searchsorted(key, np.arange(NCHUNK * nblk), side="right")

        idx = np.zeros(total, np.int64)
        dstl = np.full(total, -1.0, np.float64)
        degdst = np.ones(total, np.float64)
        for c in range(NCHUNK):
            for b in range(nblk):
                s, e = starts[c * nblk + b], ends[c * nblk + b]
                nn = e - s
                o = offs[c, b]
                assert nn <= cap[c, b]
                idx[o:o + nn] = r[s:e] % chunk
                idx[o + nn:o + cap[c, b]] = r[e - 1] % chunk if nn else 0
                dstl[o:o + nn] = dl[s:e]
                degdst[o:o + nn] = deg[dd[s:e]]
        cores.append(dict(idx=idx.astype(np.int16),
                          dstl=dstl.astype(f16),
                          degdst=degdst.astype(np.float32)))

    return deg, cap, offs, calls, total, cores


# --------------------------------------------------------------- bass build


def _build_program(cfg, cap, offs, calls, total):
    import concourse.bacc as bacc
    import concourse.tile as tile
    from concourse import mybir

    nblk, shpad, nfull, chunk = (cfg[k] for k in
                                 ("nblk", "shpad", "nfull", "chunk"))
    dt = mybir.dt
    AF = mybir.ActivationFunctionType
    OP = mybir.AluOpType
    S_all = total // 128
    idxcols = total // 16
    groups = _groups(nblk)

    nc = bacc.Bacc("TRN2", target_bir_lowering=False, debug=False,
                   num_devices=NC, num_swdge_queues=4)

    # --- I/O
    xT_d = nc.dram_tensor("xT", [N_FEAT, shpad], dt.float16, kind="ExternalInput")
    W1_d = nc.dram_tensor("W1", [N_FEAT, D], dt.float16, kind="ExternalInput")
    W2_d = nc.dram_tensor("W2", [D, D], dt.float16, kind="ExternalInput")
    W3_d = nc.dram_tensor("W3", [D, D], dt.float16, kind="ExternalInput")
    b1_d = nc.dram_tensor("b1", [D, 1], dt.float32, kind="ExternalInput")
    b2_d = nc.dram_tensor("b2", [D, 1], dt.float32, kind="ExternalInput")
    b3r_d = nc.dram_tensor("b3r", [1, D], dt.float16, kind="ExternalInput")
    deg_d = nc.dram_tensor("degc", [128, nblk], dt.float32, kind="ExternalInput")
    degdst_d = nc.dram_tensor("degdst", [128, S_all], dt.float32, kind="ExternalInput")
    dstl_d = nc.dram_tensor("dstl", [128, S_all], dt.float32, kind="ExternalInput")
    idx_d = nc.dram_tensor("idx16", [128, idxcols], dt.int16, kind="ExternalInput")
    iota_d = nc.dram_tensor("iota", [128, D], dt.float16, kind="ExternalInput")
    ones_d = nc.dram_tensor("ones1", [1, D], dt.float16, kind="ExternalInput")
    out_d = nc.dram_tensor("out", [shpad, D], dt.float32, kind="ExternalOutput")

    # internal DRAM: allgather bounce + double-buffered replicated T'
    tloc = nc.dram_tensor("t_loc", [shpad, D], dt.float16)
    tfull = [nc.dram_tensor(f"t_full{i}", [nfull, D], dt.float16) for i in range(2)]

    from contextlib import ExitStack
    with tile.TileContext(nc) as tc, ExitStack() as stack:
        # ---- resident tiles (pool stays open for the whole program)
        res = stack.enter_context(tc.tile_pool(name="res", bufs=1))
        with tc.tile_pool(name="scr", bufs=1) as scr:
            idx_sb = res.tile([128, idxcols], dt.int16, tag="idx")
            dstl_sb = res.tile([128, S_all], dt.float32, tag="dstl")
            disdst_sb = res.tile([128, S_all], dt.float32, tag="disdst")
            disc_sb = res.tile([128, nblk], dt.float32, tag="disc")
            iota_sb = res.tile([128, D], dt.float16, tag="iota")
            ones_sb = res.tile([1, D], dt.float16, tag="ones")
            xT_sb = res.tile([N_FEAT, shpad], dt.float16, tag="xT")
            W1_sb = res.tile([N_FEAT, D], dt.float16, tag="W1")
            W2_sb = res.tile([D, D], dt.float16, tag="W2")
            W3_sb = res.tile([D, D], dt.float16, tag="W3")
            b1_sb = res.tile([D, 1], dt.float32, tag="b1")
            b2_sb = res.tile([D, 1], dt.float32, tag="b2")
            b3r_sb = res.tile([1, D], dt.float16, tag="b3r")

            for sb, d in ((idx_sb, idx_d), (dstl_sb, dstl_d), (iota_sb, iota_d),
                          (ones_sb, ones_d), (xT_sb, xT_d), (W1_sb, W1_d),
                          (W2_sb, W2_d), (W3_sb, W3_d), (b1_sb, b1_d),
                          (b2_sb, b2_d), (b3r_sb, b3r_d)):
                nc.sync.dma_start(out=sb[:], in_=d[:, :])

            # dis = sqrt(1/deg) (Rsqrt activation is banned for accuracy)
            degt = scr.tile([128, S_all], dt.float32, tag="degt")
            nc.sync.dma_start(out=degt[:], in_=degdst_d[:, :])
            rec = scr.tile([128, S_all], dt.float32, tag="rec")
            nc.vector.reciprocal(rec[:], degt[:])
            nc.scalar.activation(disdst_sb[:], rec[:], AF.Sqrt)

            degc = scr.tile([128, nblk], dt.float32, tag="degc")
            nc.sync.dma_start(out=degc[:], in_=deg_d[:, :])
            recc = scr.tile([128, nblk], dt.float32, tag="recc")
            nc.vector.reciprocal(recc[:], degc[:])
            nc.scalar.activation(disc_sb[:], recc[:], AF.Sqrt)

        # ---- layer 1 transform: T1' = dis * (x @ W1) -> tloc, allgather
        with (
            tc.tile_pool(name="p1ps", bufs=4, space="PSUM") as p1ps,
            tc.tile_pool(name="p1sb", bufs=4) as p1sb,
        ):
            for b in range(nblk):
                ps = p1ps.tile([128, D], dt.float32, tag="t1ps")
                nc.tensor.matmul(ps[:], xT_sb[:, b * 128:(b + 1) * 128],
                                 W1_sb[:], start=True, stop=True)
                t1 = p1sb.tile([128, D], dt.float16, tag="t1sb")
                nc.scalar.activation(t1[:], ps[:], AF.Copy,
                                     scale=disc_sb[:, b:b + 1])
                nc.sync.dma_start(out=tloc[b * 128:(b + 1) * 128, :], in_=t1[:])
        nc.gpsimd.collective_compute(
            "AllGather", mybir.AluOpType.bypass,
            replica_groups=[list(range(NC))],
            ins=[tloc[:, :].opt()], outs=[tfull[0][:, :].opt()])

        # ---- layers
        gc_size = {}
        for (c, gi, co, ns) in calls:
            k = (gi, c)
            gc_size[k] = gc_size.get(k, 0) + ns
        maxsub = {c: max(v for (gi, cc), v in gc_size.items() if cc == c) // 128
                  for c in range(NCHUNK)}
        for layer in range(3):
            last = layer == 2
            tsrc = tfull[layer % 2]
            W_next = W2_sb if layer == 0 else W3_sb
            bias = (b1_sb, b2_sb, None)[layer]
            with (
                tc.tile_pool(name=f"msg{layer}", bufs=2) as msgp,
                tc.tile_pool(name=f"oh{layer}", bufs=8) as ohp,
                tc.tile_pool(name=f"ev{layer}", bufs=4) as evp,
                tc.tile_pool(name=f"ps{layer}", bufs=4, space="PSUM") as psp,
                tc.tile_pool(name=f"ps2{layer}", bufs=2, space="PSUM") as ps2p,
            ):
                for gi, (blo, bhi) in enumerate(groups):
                    mtiles = {}
                    for c in range(NCHUNK):
                        gcalls = [cl for cl in calls if cl[0] == c and cl[1] == gi]
                        region_off = gcalls[0][2]
                        mt = msgp.tile([128, maxsub[c] * 128], dt.float16,
                                       tag=f"msg{c}")
                        src_view = tsrc[c * chunk:(c + 1) * chunk, :]
                        for (_, _, call_off, nslots) in gcalls:
                            nsub = nslots // 128
                            fo = (call_off - region_off) // 128
                            nc.gpsimd.dma_gather(
                                mt[:, fo * 128:(fo + nsub) * 128]
                                .rearrange("p (s e) -> p s e", e=D),
                                src_view,
                                idx_sb[:, call_off // 16:(call_off + nslots) // 16],
                                nslots, nslots, D, queue_num=c)
                        mtiles[c] = (mt, region_off)
                    for b in range(blo, bhi):
                        ps = psp.tile([128, D], dt.float32, tag="ps")
                        nmm = sum(int(cap[c, b]) for c in range(NCHUNK)) // 128
                        k = 0
                        for c in range(NCHUNK):
                            mt, call_off = mtiles[c]
                            base = (int(offs[c, b]) - call_off) // 128
                            for s in range(int(cap[c, b]) // 128):
                                gs = int(offs[c, b]) // 128 + s
                                oh = ohp.tile([128, D], dt.float16, tag="oh")
                                nc.vector.tensor_scalar(
                                    oh[:], iota_sb[:],
                                    dstl_sb[:, gs:gs + 1],
                                    disdst_sb[:, gs:gs + 1],
                                    OP.is_equal, OP.mult)
                                msl = mt[:, (base + s) * 128:(base + s + 1) * 128]
                                if last:
                                    nc.tensor.matmul(ps[:], oh[:], msl,
                                                     start=(k == 0), stop=False)
                                else:
                                    nc.tensor.matmul(ps[:], msl, oh[:],
                                                     start=(k == 0),
                                                     stop=(k == nmm - 1))
                                k += 1
                        if last:
                            nc.tensor.matmul(ps[:], ones_sb[:], b3r_sb[:],
                                             start=False, stop=True)
                            ot = evp.tile([128, D], dt.float32, tag="outsb")
                            nc.scalar.activation(ot[:], ps[:], AF.Copy)
                            nc.sync.dma_start(
                                out=out_d[b * 128:(b + 1) * 128, :], in_=ot[:])
                        else:
                            hT = evp.tile([128, D], dt.float16, tag="hT")
                            nc.scalar.activation(hT[:], ps[:], AF.Relu,
                                                 bias=bias[:])
                            ps2 = ps2p.tile([128, D], dt.float32, tag="ps2")
                            nc.tensor.matmul(ps2[:], hT[:], W_next[:],
                                             start=True, stop=True)
                            tn = evp.tile([128, D], dt.float16, tag="tn")
                            nc.scalar.activation(tn[:], ps2[:], AF.Copy,
                                                 scale=disc_sb[:, b:b + 1])
                            nc.sync.dma_start(
                                out=tloc[b * 128:(b + 1) * 128, :], in_=tn[:])
                if not last:
                    nc.gpsimd.collective_compute(
                        "AllGather", mybir.AluOpType.bypass,
                        replica_groups=[list(range(NC))],
                        ins=[tloc[:, :].opt()],
                        outs=[tfull[(layer + 1) % 2][:, :].opt()])

    nc.compile()
    return nc


# ------------------------------------------------------------------ driver


def _make_in_maps(cfg, deg, cores, inputs):
    n, nshard, shpad, nblk = (cfg[k] for k in ("n", "nshard", "shpad", "nblk"))
    x = np.asarray(inputs["x"], np.float32)
    W1 = np.asarray(inputs["W1"], f16)
    W2 = np.asarray(inputs["W2"], f16)
    W3 = np.asarray(inputs["W3"], f16)
    b1 = np.asarray(inputs["b1"], np.float32).reshape(D, 1)
    b2 = np.asarray(inputs["b2"], np.float32).reshape(D, 1)
    b3r = np.asarray(inputs["b3"], f16).reshape(1, D)
    iota = np.broadcast_to(np.arange(D, dtype=f16), (128, D)).copy()
    ones1 = np.ones((1, D), f16)

    in_maps = []
    for ci in range(NC):
        xs = np.zeros((shpad, N_FEAT), np.float32)
        lo = min(ci * nshard, n)
        hi = min((ci + 1) * nshard, n)
        xs[:hi - lo] = x[lo:hi]
        degs = np.ones(shpad, np.float32)
        degs[:hi - lo] = deg[lo:hi]
        ca = cores[ci]
        total = len(ca["idx"])
        in_maps.append({
            "xT": np.ascontiguousarray(xs.T.astype(f16)),
            "W1": W1, "W2": W2, "W3": W3, "b1": b1, "b2": b2, "b3r": b3r,
            "degc": np.ascontiguousarray(degs.reshape(nblk, 128).T),
            "degdst": np.ascontiguousarray(
                ca["degdst"].reshape(total // 128, 128).T),
            "dstl": np.ascontiguousarray(
                ca["dstl"].reshape(total // 128, 128).T.astype(np.float32)),
            "idx16": np.ascontiguousarray(
                np.tile(ca["idx"].reshape(total // 16, 16).T, (8, 1))),
            "iota": iota, "ones1": ones1,
        })
    return in_maps


def run(inputs, n_nodes=N_NODES, trace=False):
    cfg = _cfg(n_nodes)
    edge_index = np.asarray(inputs["edge_index"]).astype(np.int64)
    deg, cap, offs, calls, total, cores = _build_schedule(cfg, edge_index)
    nc = _build_program(cfg, cap, offs, calls, total)
    in_maps = _make_in_maps(cfg, deg, cores, inputs)

    from concourse.bass_utils import run_bass_kernel_spmd
    res = run_bass_kernel_spmd(nc, in_maps, core_ids=list(range(NC)),
                               trace=trace)
    n, nshard = cfg["n"], cfg["nshard"]
    out = np.concatenate(
        [res.results[ci]["out"][:min((ci + 1) * nshard, n) - ci * nshard]
         for ci in range(NC)], axis=0)
    return out.astype(np.float32), res


def kernel(**inputs) -> np.ndarray:
    out, _ = run(inputs)
    return out



# revision 5
# speedup vs baseline: 1.7789x; 1.7789x over previous
"""GCN encoder (3x GCNConv) Trainium2 Bass kernel, 8-core SPMD.

Strategy (dst-sharded message passing, v2):
- Nodes dst-sharded across 8 cores (12544-row padded shards). Each core owns
  all edges (incl. self-loops) whose dst lands in its shard.
- One-hot routing tiles oh[slot, dst] = (dst_lane one-hot) * dis[dst] are
  precomputed on HOST and streamed per group via HWDGE static DMA (no
  VectorE work). dis[src] rides in the gathered T' rows.
- Layer 1 does NO gather: msg1[slot] = dis[src]*x[src] is only 4 wide and
  depends only on host-known x/edge_index, so it is inlined as an input.
  Aggregate in 4-dim space (psum[4,dst] = msg1.T @ oh), then W1/bias/relu
  and the W2 transform per dst block. Only 2 AllGathers total.
- Layers 2-3 gather T'[src] rows (fp16, replicated in DRAM via AllGather)
  with gpsimd.dma_gather (int16 indices => T_full split into 4 row-chunks).
- Layer 2 uses psum[f,d] = msg.T @ oh so the Relu+bias evac output hT is
  directly the lhsT of the next transform GEMM. Layer 3 uses
  psum[d,f] = oh.T @ msg (+ ones x b3 matmul for bias) and writes fp32
  node-major output.
"""

import sys
import numpy as np

for _p in ("/opt/trn_rl_repo", "/root/.axon_site/_ro/trn_rl_repo"):
    if _p not in sys.path:
        sys.path.append(_p)

N_NODES = 100000
N_FEAT = 4
D = 128
NC = 8
NCHUNK = 4
GBLK = 4  # blocks per gather group
MAXIDX = 1024  # max indices per dma_gather call (SWDGE desc ring capacity)

f16 = np.float16


# ---------------------------------------------------------------- host side


def _cfg(n_nodes):
    nshard = (n_nodes + NC - 1) // NC
    shpad = ((nshard + 127) // 128) * 128
    nblk = shpad // 128
    nfull = NC * shpad
    assert nfull % NCHUNK == 0
    chunk = nfull // NCHUNK
    assert chunk <= 32767 + 1  # int16 index reach (idx < chunk <= 32768)
    return dict(n=n_nodes, nshard=nshard, shpad=shpad, nblk=nblk,
                nfull=nfull, chunk=chunk)


def _groups(nblk):
    return [(g, min(g + GBLK, nblk)) for g in range(0, nblk, GBLK)]


def _build_schedule(cfg, edge_index, x):
    """Integer/index preprocessing. Returns shared capacities + per-core
    slot arrays (idx int16, oh [total,128] f16, msg1 [total,4] f16)."""
    n, nshard, shpad, nblk, chunk = (cfg[k] for k in
                                     ("n", "nshard", "shpad", "nblk", "chunk"))
    src = np.concatenate([edge_index[0], np.arange(n)]).astype(np.int64)
    dst = np.concatenate([edge_index[1], np.arange(n)]).astype(np.int64)
    deg = np.bincount(dst, minlength=n).astype(np.int64)
    dis = np.where(deg > 0, 1.0 / np.sqrt(deg.astype(np.float64)), 0.0)

    rows = (src // nshard) * shpad + (src % nshard)  # row in T_full layout
    echunk = rows // chunk
    ecore = dst // nshard
    eblk = (dst % nshard) // 128
    edstl = (dst % nshard) % 128

    counts = np.zeros((NC, NCHUNK, nblk), dtype=np.int64)
    np.add.at(counts, (ecore, echunk, eblk), 1)
    cap = counts.max(axis=0)
    cap = np.maximum(((cap + 127) // 128) * 128, 128)  # [NCHUNK, nblk]

    # static slot offsets matching emission order: group -> chunk -> block.
    # Each (group, chunk) region is subdivided into gather calls of at most
    # MAXIDX slots (SWDGE descriptor-ring capacity), never splitting a block.
    offs = np.zeros((NCHUNK, nblk), dtype=np.int64)
    calls = []  # (chunk, group_index, slot_off, nslots) per gather call
    off = 0
    for gi, (blo, bhi) in enumerate(_groups(nblk)):
        for c in range(NCHUNK):
            call_off = off
            for b in range(blo, bhi):
                if off + int(cap[c, b]) - call_off > MAXIDX:
                    calls.append((c, gi, call_off, off - call_off))
                    call_off = off
                offs[c, b] = off
                off += int(cap[c, b])
            calls.append((c, gi, call_off, off - call_off))
    calls = [cl for cl in calls if cl[3] > 0]
    total = off

    sdis = dis[src]  # dis[src[e]] per message
    ddis = dis[dst]
    xs = x.astype(np.float64)[src] * sdis[:, None]  # dis[src]*x[src] per message

    cores = []
    for ci in range(NC):
        m = ecore == ci
        r, ec, eb, dl, dd, sx = (a[m] for a in (rows, echunk, eblk, edstl,
                                                ddis, xs))
        order = np.lexsort((r, eb, ec))
        r, ec, eb, dl, dd, sx = (a[order] for a in (r, ec, eb, dl, dd, sx))
        key = ec * nblk + eb
        starts = np.searchsorted(key, np.arange(NCHUNK * nblk))
        ends = np.searchsorted(key, np.arange(NCHUNK * nblk), side="right")

        idx = np.zeros(total, np.int64)
        ohv = np.zeros(total, np.float64)   # dis[dst] value (0 => pad slot)
        ohl = np.zeros(total, np.int64)     # dst lane
        msg1 = np.zeros((total, N_FEAT), np.float64)
        for c in range(NCHUNK):
            for b in range(nblk):
                s, e = starts[c * nblk + b], ends[c * nblk + b]
                nn = e - s
                o = offs[c, b]
                assert nn <= cap[c, b]
                idx[o:o + nn] = r[s:e] % chunk
                idx[o + nn:o + cap[c, b]] = r[e - 1] % chunk if nn else 0
                ohl[o:o + nn] = dl[s:e]
                ohv[o:o + nn] = dd[s:e]
                msg1[o:o + nn] = sx[s:e]
        oh = np.zeros((total, D), f16)
        oh[np.arange(total), ohl] = ohv.astype(f16)
        cores.append(dict(idx=idx.astype(np.int16), oh=oh,
                          msg1=msg1.astype(f16)))

    return deg, cap, offs, calls, total, cores


# --------------------------------------------------------------- bass build


def _build_program(cfg, cap, offs, calls, total):
    import concourse.bacc as bacc
    import concourse.tile as tile
    from concourse import mybir

    nblk, shpad, nfull, chunk = (cfg[k] for k in
                                 ("nblk", "shpad", "nfull", "chunk"))
    dt = mybir.dt
    AF = mybir.ActivationFunctionType
    S_all = total // 128
    idxcols = total // 16
    groups = _groups(nblk)

    nc = bacc.Bacc("TRN2", target_bir_lowering=False, debug=False,
                   num_devices=NC, num_swdge_queues=4)

    # --- I/O
    W1_d = nc.dram_tensor("W1", [N_FEAT, D], dt.float16, kind="ExternalInput")
    W2_d = nc.dram_tensor("W2", [D, D], dt.float16, kind="ExternalInput")
    W3_d = nc.dram_tensor("W3", [D, D], dt.float16, kind="ExternalInput")
    b1_d = nc.dram_tensor("b1", [D, 1], dt.float32, kind="ExternalInput")
    b2_d = nc.dram_tensor("b2", [D, 1], dt.float32, kind="ExternalInput")
    b3r_d = nc.dram_tensor("b3r", [1, D], dt.float16, kind="ExternalInput")
    deg_d = nc.dram_tensor("degc", [128, nblk], dt.float32, kind="ExternalInput")
    idx_d = nc.dram_tensor("idx16", [128, idxcols], dt.int16, kind="ExternalInput")
    oh_d = nc.dram_tensor("ohsw", [128, S_all * D], dt.float16, kind="ExternalInput")
    m1_d = nc.dram_tensor("m1sw", [128, S_all * N_FEAT], dt.float16,
                          kind="ExternalInput")
    ones_d = nc.dram_tensor("ones1", [1, D], dt.float16, kind="ExternalInput")
    out_d = nc.dram_tensor("out", [shpad, D], dt.float32, kind="ExternalOutput")

    # internal DRAM: allgather bounce + double-buffered replicated T'
    tloc = nc.dram_tensor("t_loc", [shpad, D], dt.float16)
    tfull = [nc.dram_tensor(f"t_full{i}", [nfull, D], dt.float16) for i in range(2)]

    # group g covers slot-blocks [gsb0[g], gsb0[g+1]) (contiguous emission)
    gsb0 = []
    for gi, (blo, bhi) in enumerate(groups):
        gsb0.append(int(min(offs[c, blo] for c in range(NCHUNK))) // 128)
    gsb0.append(S_all)

    from contextlib import ExitStack
    with tile.TileContext(nc) as tc, ExitStack() as stack:
        # ---- resident tiles (pool stays open for the whole program)
        res = stack.enter_context(tc.tile_pool(name="res", bufs=1))
        with tc.tile_pool(name="scr", bufs=1) as scr:
            idx_sb = res.tile([128, idxcols], dt.int16, tag="idx")
            disc_sb = res.tile([128, nblk], dt.float32, tag="disc")
            ones_sb = res.tile([1, D], dt.float16, tag="ones")
            W1_sb = res.tile([N_FEAT, D], dt.float16, tag="W1")
            W2_sb = res.tile([D, D], dt.float16, tag="W2")
            W3_sb = res.tile([D, D], dt.float16, tag="W3")
            b1_sb = res.tile([D, 1], dt.float32, tag="b1")
            b2_sb = res.tile([D, 1], dt.float32, tag="b2")
            b3r_sb = res.tile([1, D], dt.float16, tag="b3r")

            for sb, d in ((idx_sb, idx_d), (ones_sb, ones_d), (W1_sb, W1_d),
                          (W2_sb, W2_d), (W3_sb, W3_d), (b1_sb, b1_d),
                          (b2_sb, b2_d), (b3r_sb, b3r_d)):
                nc.sync.dma_start(out=sb[:], in_=d[:, :])

            # dis = sqrt(1/deg) (Rsqrt activation is banned for accuracy)
            degc = scr.tile([128, nblk], dt.float32, tag="degc")
            nc.sync.dma_start(out=degc[:], in_=deg_d[:, :])
            recc = scr.tile([128, nblk], dt.float32, tag="recc")
            nc.vector.reciprocal(recc[:], degc[:])
            nc.scalar.activation(disc_sb[:], recc[:], AF.Sqrt)

        # ---- layers
        gc_size = {}
        for (c, gi, co, ns) in calls:
            k = (gi, c)
            gc_size[k] = gc_size.get(k, 0) + ns
        maxsub = {c: max(v for (gi, cc), v in gc_size.items() if cc == c) // 128
                  for c in range(NCHUNK)}
        for layer in range(3):
            first = layer == 0
            last = layer == 2
            tsrc = tfull[(layer + 1) % 2]
            W_next = W2_sb if layer < 2 else None
            bias = (b1_sb, b2_sb, None)[layer]
            with (
                tc.tile_pool(name=f"msg{layer}", bufs=2) as msgp,
                tc.tile_pool(name=f"oh{layer}", bufs=2) as ohp,
                tc.tile_pool(name=f"ev{layer}", bufs=4) as evp,
                tc.tile_pool(name=f"ps{layer}", bufs=4, space="PSUM") as psp,
                tc.tile_pool(name=f"ps2{layer}", bufs=2, space="PSUM") as ps2p,
            ):
                for gi, (blo, bhi) in enumerate(groups):
                    nsb = gsb0[gi + 1] - gsb0[gi]
                    ohT = ohp.tile([128, maxsub_g(maxsub) * D], dt.float16,
                                   tag="ohg")
                    nc.sync.dma_start(
                        out=ohT[:, :nsb * D],
                        in_=oh_d[:, gsb0[gi] * D:gsb0[gi + 1] * D])
                    if first:
                        m1T = msgp.tile([128, maxsub_g(maxsub) * N_FEAT],
                                        dt.float16, tag="m1g")
                        nc.sync.dma_start(
                            out=m1T[:, :nsb * N_FEAT],
                            in_=m1_d[:, gsb0[gi] * N_FEAT:gsb0[gi + 1] * N_FEAT])
                        mtiles = None
                    else:
                        mtiles = {}
                        for c in range(NCHUNK):
                            gcalls = [cl for cl in calls
                                      if cl[0] == c and cl[1] == gi]
                            region_off = gcalls[0][2]
                            mt = msgp.tile([128, maxsub[c] * 128], dt.float16,
                                           tag=f"msg{c}")
                            src_view = tsrc[c * chunk:(c + 1) * chunk, :]
                            for (_, _, call_off, nslots) in gcalls:
                                nsub = nslots // 128
                                fo = (call_off - region_off) // 128
                                nc.gpsimd.dma_gather(
                                    mt[:, fo * 128:(fo + nsub) * 128]
                                    .rearrange("p (s e) -> p s e", e=D),
                                    src_view,
                                    idx_sb[:, call_off // 16:(call_off + nslots) // 16],
                                    nslots, nslots, D, queue_num=c)
                            mtiles[c] = (mt, region_off)
                    for b in range(blo, bhi):
                        ps = psp.tile([N_FEAT if first else 128, D],
                                      dt.float32, tag="ps")
                        nmm = sum(int(cap[c, b]) for c in range(NCHUNK)) // 128
                        k = 0
                        for c in range(NCHUNK):
                            if first:
                                base = int(offs[c, b]) // 128 - gsb0[gi]
                            else:
                                mt, call_off = mtiles[c]
                                base = (int(offs[c, b]) - call_off) // 128
                            for s in range(int(cap[c, b]) // 128):
                                gs = int(offs[c, b]) // 128 + s - gsb0[gi]
                                oh = ohT[:, gs * D:(gs + 1) * D]
                                if first:
                                    msl = m1T[:, (base + s) * N_FEAT:
                                              (base + s + 1) * N_FEAT]
                                else:
                                    msl = mt[:, (base + s) * 128:
                                             (base + s + 1) * 128]
                                if last:
                                    nc.tensor.matmul(ps[:], oh, msl,
                                                     start=(k == 0), stop=False)
                                else:
                                    nc.tensor.matmul(ps[:], msl, oh,
                                                     start=(k == 0),
                                                     stop=(k == nmm - 1))
                                k += 1
                        if last:
                            nc.tensor.matmul(ps[:], ones_sb[:], b3r_sb[:],
                                             start=False, stop=True)
                            ot = evp.tile([128, D], dt.float32, tag="outsb")
                            nc.scalar.activation(ot[:], ps[:], AF.Copy)
                            nc.sync.dma_start(
                                out=out_d[b * 128:(b + 1) * 128, :], in_=ot[:])
                        else:
                            if first:
                                # agg4[4,dst] -> psH2[f,dst] = W1.T @ t4
                                t4 = evp.tile([N_FEAT, D], dt.float16, tag="t4")
                                nc.scalar.activation(t4[:], ps[:], AF.Copy)
                                psH = ps2p.tile([128, D], dt.float32, tag="psH")
                                nc.tensor.matmul(psH[:], W1_sb[:], t4[:],
                                                 start=True, stop=True)
                                ps_ev = psH
                            else:
                                ps_ev = ps
                            hT = evp.tile([128, D], dt.float16, tag="hT")
                            nc.scalar.activation(hT[:], ps_ev[:], AF.Relu,
                                                 bias=bias[:])
                            ps2 = ps2p.tile([128, D], dt.float32, tag="ps2")
                            nc.tensor.matmul(ps2[:], hT[:],
                                             W2_sb if first else W3_sb,
                                             start=True, stop=True)
                            tn = evp.tile([128, D], dt.float16, tag="tn")
                            nc.scalar.activation(tn[:], ps2[:], AF.Copy,
                                                 scale=disc_sb[:, b:b + 1])
                            nc.sync.dma_start(
                                out=tloc[b * 128:(b + 1) * 128, :], in_=tn[:])
                if not last:
                    nc.gpsimd.collective_compute(
                        "AllGather", mybir.AluOpType.bypass,
                        replica_groups=[list(range(NC))],
                        ins=[tloc[:, :].opt()],
                        outs=[tfull[layer % 2][:, :].opt()])

    nc.compile()
    return nc


def maxsub_g(maxsub):
    return sum(maxsub.values())


# ------------------------------------------------------------------ driver


def _make_in_maps(cfg, deg, cores, inputs, total):
    n, nshard, shpad, nblk = (cfg[k] for k in ("n", "nshard", "shpad", "nblk"))
    W1 = np.asarray(inputs["W1"], f16)
    W2 = np.asarray(inputs["W2"], f16)
    W3 = np.asarray(inputs["W3"], f16)
    b1 = np.asarray(inputs["b1"], np.float32).reshape(D, 1)
    b2 = np.asarray(inputs["b2"], np.float32).reshape(D, 1)
    b3r = np.asarray(inputs["b3"], f16).reshape(1, D)
    ones1 = np.ones((1, D), f16)
    S_all = total // 128

    in_maps = []
    for ci in range(NC):
        lo = min(ci * nshard, n)
        hi = min((ci + 1) * nshard, n)
        degs = np.ones(shpad, np.float32)
        degs[:hi - lo] = deg[lo:hi]
        ca = cores[ci]
        ohsw = np.ascontiguousarray(
            ca["oh"].reshape(S_all, 128, D).transpose(1, 0, 2)
            .reshape(128, S_all * D))
        m1sw = np.ascontiguousarray(
            ca["msg1"].reshape(S_all, 128, N_FEAT).transpose(1, 0, 2)
            .reshape(128, S_all * N_FEAT))
        in_maps.append({
            "W1": W1, "W2": W2, "W3": W3, "b1": b1, "b2": b2, "b3r": b3r,
            "degc": np.ascontiguousarray(degs.reshape(nblk, 128).T),
            "idx16": np.ascontiguousarray(
                np.tile(ca["idx"].reshape(total // 16, 16).T, (8, 1))),
            "ohsw": ohsw, "m1sw": m1sw, "ones1": ones1,
        })
    return in_maps


def run(inputs, n_nodes=N_NODES, trace=False):
    cfg = _cfg(n_nodes)
    edge_index = np.asarray(inputs["edge_index"]).astype(np.int64)
    x = np.asarray(inputs["x"], np.float32)
    deg, cap, offs, calls, total, cores = _build_schedule(cfg, edge_index, x)
    nc = _build_program(cfg, cap, offs, calls, total)
    in_maps = _make_in_maps(cfg, deg, cores, inputs, total)

    from concourse.bass_utils import run_bass_kernel_spmd
    res = run_bass_kernel_spmd(nc, in_maps, core_ids=list(range(NC)),
                               trace=trace)
    n, nshard = cfg["n"], cfg["nshard"]
    out = np.concatenate(
        [res.results[ci]["out"][:min((ci + 1) * nshard, n) - ci * nshard]
         for ci in range(NC)], axis=0)
    return out.astype(np.float32), res


def kernel(**inputs) -> np.ndarray:
    out, _ = run(inputs)
    return out


# revision 15
# speedup vs baseline: 2.2591x; 1.2699x over previous
"""GCN encoder (3x GCNConv) Trainium2 Bass kernel, 8-core SPMD.

Strategy (dst-sharded message passing, v2):
- Nodes dst-sharded across 8 cores (12544-row padded shards). Each core owns
  all edges (incl. self-loops) whose dst lands in its shard.
- One-hot routing tiles oh[slot, dst] = (dst_lane one-hot) * dis[dst] are
  precomputed on HOST and streamed per group via HWDGE static DMA (no
  VectorE work). dis[src] rides in the gathered T' rows.
- Layer 1 does NO gather: msg1[slot] = dis[src]*x[src] is only 4 wide and
  depends only on host-known x/edge_index, so it is inlined as an input.
  Aggregate in 4-dim space (psum[4,dst] = msg1.T @ oh), then W1/bias/relu
  and the W2 transform per dst block. Only 2 AllGathers total.
- Layers 2-3 gather T'[src] rows (fp16, replicated in DRAM via AllGather)
  with gpsimd.dma_gather (int16 indices => T_full split into 4 row-chunks).
- Layer 2 uses psum[f,d] = msg.T @ oh so the Relu+bias evac output hT is
  directly the lhsT of the next transform GEMM. Layer 3 uses
  psum[d,f] = oh.T @ msg (+ ones x b3 matmul for bias) and writes fp32
  node-major output.
"""

import sys
import numpy as np

for _p in ("/opt/trn_rl_repo", "/root/.axon_site/_ro/trn_rl_repo"):
    if _p not in sys.path:
        sys.path.append(_p)

N_NODES = 100000
N_FEAT = 4
D = 128
NC = 8
NCHUNK = 4
GBLK = 4  # blocks per gather group
MAXIDX = 1024  # max indices per dma_gather call (SWDGE desc ring capacity)

f16 = np.float16


# ---------------------------------------------------------------- host side


def _cfg(n_nodes):
    nshard = (n_nodes + NC - 1) // NC
    shpad = ((nshard + 127) // 128) * 128
    nblk = shpad // 128
    nfull = NC * shpad
    assert nfull % NCHUNK == 0
    chunk = nfull // NCHUNK
    assert chunk <= 32767 + 1  # int16 index reach (idx < chunk <= 32768)
    return dict(n=n_nodes, nshard=nshard, shpad=shpad, nblk=nblk,
                nfull=nfull, chunk=chunk)


def _groups(nblk):
    return [(g, min(g + GBLK, nblk)) for g in range(0, nblk, GBLK)]


def _balance(n, w, nbins):
    """LPT: assign nodes to nbins bins of capacity 128, balancing total
    weight per bin. Returns pos[v] = bin*128 + lane."""
    import heapq
    order = np.argsort(-w, kind="stable")
    heap = [(0, b) for b in range(nbins)]
    heapq.heapify(heap)
    fill = np.zeros(nbins, np.int64)
    pos = np.empty(n, np.int64)
    for v in order:
        while True:
            load, b = heapq.heappop(heap)
            if fill[b] < 128:
                break
        pos[v] = b * 128 + fill[b]
        fill[b] += 1
        if fill[b] < 128:
            heapq.heappush(heap, (load + int(w[v]), b))
    return pos


def _build_schedule(cfg, edge_index, x):
    """Integer/index preprocessing. Returns shared capacities + per-core
    slot arrays (idx int16, oh [total,128] f16, msg1 [total,4] f16).
    Node ids are permuted (pos) to balance per-(core,block) message counts;
    self-loops get one dedicated slot-block per dst block (loaded from tloc
    by static DMA instead of gathered)."""
    n, nshard, shpad, nblk, chunk = (cfg[k] for k in
                                     ("n", "nshard", "shpad", "nblk", "chunk"))
    es, ed = edge_index[0].astype(np.int64), edge_index[1].astype(np.int64)
    deg = (np.bincount(ed, minlength=n) + 1).astype(np.int64)  # incl self
    dis = 1.0 / np.sqrt(deg.astype(np.float64))

    indeg = np.bincount(ed, minlength=n)
    pos = _balance(n, indeg, NC * nblk)  # new node id == T_full row

    rows = pos[es]                      # gather row of each message's src
    ndst = pos[ed]
    echunk = rows // chunk
    ecore = ndst // shpad
    eblk = (ndst % shpad) // 128
    edstl = ndst % 128

    counts = np.zeros((NC, NCHUNK, nblk), dtype=np.int64)
    np.add.at(counts, (ecore, echunk, eblk), 1)
    cap = counts.max(axis=0)
    cap = np.maximum(((cap + 127) // 128) * 128, 128)  # [NCHUNK, nblk]

    # static slot offsets matching emission order: group -> chunk -> block
    # -> one self slot-block per block. Each (group, chunk) region is
    # subdivided into gather calls of at most MAXIDX slots (SWDGE
    # descriptor-ring capacity), never splitting a block.
    offs = np.zeros((NCHUNK, nblk), dtype=np.int64)
    selfoff = np.zeros(nblk, dtype=np.int64)
    calls = []  # (chunk, group_index, slot_off, nslots) per gather call
    off = 0
    for gi, (blo, bhi) in enumerate(_groups(nblk)):
        for c in range(NCHUNK):
            call_off = off
            for b in range(blo, bhi):
                if off + int(cap[c, b]) - call_off > MAXIDX:
                    calls.append((c, gi, call_off, off - call_off))
                    call_off = off
                offs[c, b] = off
                off += int(cap[c, b])
            calls.append((c, gi, call_off, off - call_off))
        for b in range(blo, bhi):
            selfoff[b] = off
            off += 128
    calls = [cl for cl in calls if cl[3] > 0]
    total = off

    sdis = dis[es]  # dis[src[e]] per message
    ddis = dis[ed]
    xs = x.astype(np.float64)[es] * sdis[:, None]  # dis[src]*x[src]
    xself = x.astype(np.float64) * dis[:, None]

    cores = []
    for ci in range(NC):
        m = ecore == ci
        r, ec, eb, dl, dd, sx = (a[m] for a in (rows, echunk, eblk, edstl,
                                                ddis, xs))
        order = np.lexsort((r, eb, ec))
        r, ec, eb, dl, dd, sx = (a[order] for a in (r, ec, eb, dl, dd, sx))
        key = ec * nblk + eb
        starts = np.searchsorted(key, np.arange(NCHUNK * nblk))
        ends = np.searchsorted(key, np.arange(NCHUNK * nblk), side="right")

        idx = np.zeros(total, np.int64)
        ohv = np.zeros(total, np.float64)   # dis[dst] value (0 => pad slot)
        ohl = np.zeros(total, np.int64)     # dst lane
        msg1 = np.zeros((total, N_FEAT), np.float64)
        for c in range(NCHUNK):
            for b in range(nblk):
                s, e = starts[c * nblk + b], ends[c * nblk + b]
                nn = e - s
                o = offs[c, b]
                assert nn <= cap[c, b]
                idx[o:o + nn] = r[s:e] % chunk
                idx[o + nn:o + cap[c, b]] = r[e - 1] % chunk if nn else 0
                ohl[o:o + nn] = dl[s:e]
                ohv[o:o + nn] = dd[s:e]
                msg1[o:o + nn] = sx[s:e]
        # self-loop slot-blocks: lane l routes to dst lane l with dis[v]
        mine = (pos // shpad) == ci
        lpos = pos[mine] % shpad
        o = selfoff[lpos // 128] + lpos % 128
        ohl[o] = lpos % 128
        ohv[o] = dis[mine]
        msg1[o] = xself[mine]
        oh = np.zeros((total, D), f16)
        oh[np.arange(total), ohl] = ohv.astype(f16)
        cores.append(dict(idx=idx.astype(np.int16), oh=oh,
                          msg1=msg1.astype(f16)))

    return deg, dis, pos, cap, offs, selfoff, calls, total, cores


# --------------------------------------------------------------- bass build


def _build_program(cfg, cap, offs, selfoff, calls, total):
    import concourse.bacc as bacc
    import concourse.tile as tile
    from concourse import mybir

    nblk, shpad, nfull, chunk = (cfg[k] for k in
                                 ("nblk", "shpad", "nfull", "chunk"))
    dt = mybir.dt
    AF = mybir.ActivationFunctionType
    S_all = total // 128
    idxcols = total // 16
    groups = _groups(nblk)

    nc = bacc.Bacc("TRN2", target_bir_lowering=False, debug=False,
                   num_devices=NC, num_swdge_queues=4)

    # --- I/O
    W1_d = nc.dram_tensor("W1", [N_FEAT, D], dt.float16, kind="ExternalInput")
    W2_d = nc.dram_tensor("W2", [D, D], dt.float16, kind="ExternalInput")
    W3_d = nc.dram_tensor("W3", [D, D], dt.float16, kind="ExternalInput")
    b1_d = nc.dram_tensor("b1", [D, 1], dt.float32, kind="ExternalInput")
    b2_d = nc.dram_tensor("b2", [D, 1], dt.float32, kind="ExternalInput")
    b3r_d = nc.dram_tensor("b3r", [1, D], dt.float16, kind="ExternalInput")
    deg_d = nc.dram_tensor("degc", [128, nblk], dt.float32, kind="ExternalInput")
    idx_d = nc.dram_tensor("idx16", [128, idxcols], dt.int16, kind="ExternalInput")
    oh_d = nc.dram_tensor("ohsw", [128, S_all * D], dt.float16, kind="ExternalInput")
    m1_d = nc.dram_tensor("m1sw", [128, S_all * N_FEAT], dt.float16,
                          kind="ExternalInput")
    ones_d = nc.dram_tensor("ones1", [1, D], dt.float16, kind="ExternalInput")
    out_d = nc.dram_tensor("out", [shpad, D], dt.float32, kind="ExternalOutput")

    # internal DRAM: allgather bounce + double-buffered replicated T'
    tloc = nc.dram_tensor("t_loc", [shpad, D], dt.float16)
    tfull = [nc.dram_tensor(f"t_full{i}", [nfull, D], dt.float16) for i in range(2)]

    # group g covers slot-blocks [gsb0[g], gsb0[g+1]) (contiguous emission)
    gsb0 = []
    for gi, (blo, bhi) in enumerate(groups):
        gsb0.append(int(min(offs[c, blo] for c in range(NCHUNK))) // 128)
    gsb0.append(S_all)

    from contextlib import ExitStack
    with tile.TileContext(nc) as tc, ExitStack() as stack:
        # ---- resident tiles (pool stays open for the whole program)
        res = stack.enter_context(tc.tile_pool(name="res", bufs=1))
        with tc.tile_pool(name="scr", bufs=1) as scr:
            idx_sb = res.tile([128, idxcols], dt.int16, tag="idx")
            disc_sb = res.tile([128, nblk], dt.float32, tag="disc")
            ones_sb = res.tile([1, D], dt.float16, tag="ones")
            W1_sb = res.tile([N_FEAT, D], dt.float16, tag="W1")
            W2_sb = res.tile([D, D], dt.float16, tag="W2")
            W3_sb = res.tile([D, D], dt.float16, tag="W3")
            b1_sb = res.tile([D, 1], dt.float32, tag="b1")
            b2_sb = res.tile([D, 1], dt.float32, tag="b2")
            b3r_sb = res.tile([1, D], dt.float16, tag="b3r")

            for sb, d in ((idx_sb, idx_d), (ones_sb, ones_d), (W1_sb, W1_d),
                          (W2_sb, W2_d), (W3_sb, W3_d), (b1_sb, b1_d),
                          (b2_sb, b2_d), (b3r_sb, b3r_d)):
                nc.sync.dma_start(out=sb[:], in_=d[:, :])

            # dis = sqrt(1/deg) (Rsqrt activation is banned for accuracy)
            degc = scr.tile([128, nblk], dt.float32, tag="degc")
            nc.sync.dma_start(out=degc[:], in_=deg_d[:, :])
            recc = scr.tile([128, nblk], dt.float32, tag="recc")
            nc.vector.reciprocal(recc[:], degc[:])
            nc.scalar.activation(disc_sb[:], recc[:], AF.Sqrt)

        # ---- layers
        gc_size = {}
        for (c, gi, co, ns) in calls:
            k = (gi, c)
            gc_size[k] = gc_size.get(k, 0) + ns
        maxsub = {c: max(v for (gi, cc), v in gc_size.items() if cc == c) // 128
                  for c in range(NCHUNK)}
        for layer in range(3):
            first = layer == 0
            last = layer == 2
            tsrc = tfull[(layer + 1) % 2]
            W_next = W2_sb if layer < 2 else None
            bias = (b1_sb, b2_sb, None)[layer]
            with (
                tc.tile_pool(name=f"msg{layer}", bufs=2) as msgp,
                tc.tile_pool(name=f"oh{layer}", bufs=2) as ohp,
                tc.tile_pool(name=f"ev{layer}", bufs=4) as evp,
                tc.tile_pool(name=f"ps{layer}", bufs=4, space="PSUM") as psp,
                tc.tile_pool(name=f"ps2{layer}", bufs=2, space="PSUM") as ps2p,
            ):
                for gi, (blo, bhi) in enumerate(groups):
                    nsb = gsb0[gi + 1] - gsb0[gi]
                    ngb = bhi - blo
                    ohT = ohp.tile([128, (maxsub_g(maxsub) + GBLK) * D],
                                   dt.float16, tag="ohg")
                    nc.sync.dma_start(
                        out=ohT[:, :nsb * D],
                        in_=oh_d[:, gsb0[gi] * D:gsb0[gi + 1] * D])
                    if first:
                        m1T = msgp.tile([128, (maxsub_g(maxsub) + GBLK) * N_FEAT],
                                        dt.float16, tag="m1g")
                        nc.sync.dma_start(
                            out=m1T[:, :nsb * N_FEAT],
                            in_=m1_d[:, gsb0[gi] * N_FEAT:gsb0[gi + 1] * N_FEAT])
                        mtiles = None
                    else:
                        selfT = msgp.tile([128, GBLK * 128], dt.float16,
                                          tag="selfT")
                        nc.sync.dma_start(
                            out=selfT[:, :ngb * 128]
                            .rearrange("p (s e) -> p s e", e=D),
                            in_=tloc[blo * 128:bhi * 128, :]
                            .rearrange("(s p) e -> p s e", p=128))
                        mtiles = {}
                        for c in range(NCHUNK):
                            gcalls = [cl for cl in calls
                                      if cl[0] == c and cl[1] == gi]
                            region_off = gcalls[0][2]
                            mt = msgp.tile([128, maxsub[c] * 128], dt.float16,
                                           tag=f"msg{c}")
                            src_view = tsrc[c * chunk:(c + 1) * chunk, :]
                            for (_, _, call_off, nslots) in gcalls:
                                nsub = nslots // 128
                                fo = (call_off - region_off) // 128
                                nc.gpsimd.dma_gather(
                                    mt[:, fo * 128:(fo + nsub) * 128]
                                    .rearrange("p (s e) -> p s e", e=D),
                                    src_view,
                                    idx_sb[:, call_off // 16:(call_off + nslots) // 16],
                                    nslots, nslots, D, queue_num=c)
                            mtiles[c] = (mt, region_off)
                    for b in range(blo, bhi):
                        ps = psp.tile([N_FEAT if first else 128, D],
                                      dt.float32, tag="ps")
                        nmm = sum(int(cap[c, b]) for c in range(NCHUNK)) // 128 + 1
                        subs = []
                        for c in range(NCHUNK):
                            if first:
                                base = int(offs[c, b]) // 128 - gsb0[gi]
                                mt = None
                            else:
                                mt, call_off = mtiles[c]
                                base = (int(offs[c, b]) - call_off) // 128
                            for s in range(int(cap[c, b]) // 128):
                                gs = int(offs[c, b]) // 128 + s - gsb0[gi]
                                subs.append((gs, mt, base + s))
                        # self-loop slot-block (msg rows live in tloc)
                        subs.append((int(selfoff[b]) // 128 - gsb0[gi],
                                     None if first else selfT, b - blo))
                        for k, (gs, mt, si) in enumerate(subs):
                            oh = ohT[:, gs * D:(gs + 1) * D]
                            if first:
                                msl = m1T[:, gs * N_FEAT:(gs + 1) * N_FEAT]
                            else:
                                msl = mt[:, si * 128:(si + 1) * 128]
                            if last:
                                nc.tensor.matmul(ps[:], oh, msl,
                                                 start=(k == 0), stop=False)
                            else:
                                nc.tensor.matmul(ps[:], msl, oh,
                                                 start=(k == 0),
                                                 stop=(k == nmm - 1))
                        if last:
                            nc.tensor.matmul(ps[:], ones_sb[:], b3r_sb[:],
                                             start=False, stop=True)
                            ot = evp.tile([128, D], dt.float32, tag="outsb")
                            nc.scalar.activation(ot[:], ps[:], AF.Copy)
                            nc.sync.dma_start(
                                out=out_d[b * 128:(b + 1) * 128, :], in_=ot[:])
                        else:
                            if first:
                                # agg4[4,dst] -> psH2[f,dst] = W1.T @ t4
                                t4 = evp.tile([N_FEAT, D], dt.float16, tag="t4")
                                nc.scalar.activation(t4[:], ps[:], AF.Copy)
                                psH = ps2p.tile([128, D], dt.float32, tag="psH")
                                nc.tensor.matmul(psH[:], W1_sb[:], t4[:],
                                                 start=True, stop=True)
                                ps_ev = psH
                            else:
                                ps_ev = ps
                            hT = evp.tile([128, D], dt.float16, tag="hT")
                            nc.scalar.activation(hT[:], ps_ev[:], AF.Relu,
                                                 bias=bias[:])
                            ps2 = ps2p.tile([128, D], dt.float32, tag="ps2")
                            nc.tensor.matmul(ps2[:], hT[:],
                                             W2_sb if first else W3_sb,
                                             start=True, stop=True)
                            tn = evp.tile([128, D], dt.float16, tag="tn")
                            nc.scalar.activation(tn[:], ps2[:], AF.Copy,
                                                 scale=disc_sb[:, b:b + 1])
                            nc.sync.dma_start(
                                out=tloc[b * 128:(b + 1) * 128, :], in_=tn[:])
                if not last:
                    nc.gpsimd.collective_compute(
                        "AllGather", mybir.AluOpType.bypass,
                        replica_groups=[list(range(NC))],
                        ins=[tloc[:, :].opt()],
                        outs=[tfull[layer % 2][:, :].opt()])

    nc.compile()
    return nc


def maxsub_g(maxsub):
    return sum(maxsub.values())


# ------------------------------------------------------------------ driver


def _make_in_maps(cfg, deg, pos, cores, inputs, total):
    n, nshard, shpad, nblk = (cfg[k] for k in ("n", "nshard", "shpad", "nblk"))
    W1 = np.asarray(inputs["W1"], f16)
    W2 = np.asarray(inputs["W2"], f16)
    W3 = np.asarray(inputs["W3"], f16)
    b1 = np.asarray(inputs["b1"], np.float32).reshape(D, 1)
    b2 = np.asarray(inputs["b2"], np.float32).reshape(D, 1)
    b3r = np.asarray(inputs["b3"], f16).reshape(1, D)
    ones1 = np.ones((1, D), f16)
    S_all = total // 128
    degfull = np.ones(NC * shpad, np.float32)
    degfull[pos] = deg

    in_maps = []
    for ci in range(NC):
        degs = degfull[ci * shpad:(ci + 1) * shpad]
        ca = cores[ci]
        ohsw = np.ascontiguousarray(
            ca["oh"].reshape(S_all, 128, D).transpose(1, 0, 2)
            .reshape(128, S_all * D))
        m1sw = np.ascontiguousarray(
            ca["msg1"].reshape(S_all, 128, N_FEAT).transpose(1, 0, 2)
            .reshape(128, S_all * N_FEAT))
        in_maps.append({
            "W1": W1, "W2": W2, "W3": W3, "b1": b1, "b2": b2, "b3r": b3r,
            "degc": np.ascontiguousarray(degs.reshape(nblk, 128).T),
            "idx16": np.ascontiguousarray(
                np.tile(ca["idx"].reshape(total // 16, 16).T, (8, 1))),
            "ohsw": ohsw, "m1sw": m1sw, "ones1": ones1,
        })
    return in_maps


def run(inputs, n_nodes=N_NODES, trace=False):
    cfg = _cfg(n_nodes)
    edge_index = np.asarray(inputs["edge_index"]).astype(np.int64)
    x = np.asarray(inputs["x"], np.float32)
    (deg, dis, pos, cap, offs, selfoff, calls, total,
     cores) = _build_schedule(cfg, edge_index, x)
    nc = _build_program(cfg, cap, offs, selfoff, calls, total)
    in_maps = _make_in_maps(cfg, deg, pos, cores, inputs, total)

    from concourse.bass_utils import run_bass_kernel_spmd
    res = run_bass_kernel_spmd(nc, in_maps, core_ids=list(range(NC)),
                               trace=trace)
    stacked = np.concatenate([res.results[ci]["out"] for ci in range(NC)],
                             axis=0)
    return stacked[pos].astype(np.float32), res


def kernel(**inputs) -> np.ndarray:
    out, _ = run(inputs)
    return out
